# revision 1
# baseline (speedup 1.0000x reference)
"""Segment-softmax (GAT attention stage 4) Trainium2 kernel, 8 NeuronCores.

alpha_i = exp(e_i) / (sum_{j: tgt_j = tgt_i} exp(e_j) + 1e-16)

Strategy (edge-parallel, per the sharding hint):
  - Edges are sharded across the 8 cores (800k edges each).
  - Node index factorization t = r*128 + q with q in [0,128), r in [0,782).
  - Phase S (per core): bilinear one-hot histogram. For each chunk of 128
    edges (one SBUF column), DVE builds one-hot factor matrices
    (exp(e)*onehot_q) [128,128] and onehot_r [128,782] in fp16 via
    tensor_scalar (2x perf mode); the PE accumulates lhsT^T @ rhs into a
    PSUM-resident [128q, 782r] local table.
  - Table all-reduce: each core's local table goes to HBM; phase G reads all
    8 tables and sums them on-device (the 8-way all-reduce).
  - Phase G (per core): W = min(1/(T_sum+1e-16), 6e4) in fp16; per chunk the
    PE computes M = W^T-blocks @ onehot_r^T (a row-gather), DVE masks with
    onehot_q^T, and a ones-matmul reduces over q to yield the per-edge
    denominator reciprocal w; alpha = exp(e) * w.  The transposed one-hot
    row seeds come from host-permuted (pure data layout) q/r copies fed to
    K=1 outer-product matmuls.

The heavy lifting (histogram + gather) runs on device; the host only shards,
pads, permutes layouts, concatenates buffers between the two NEFF launches,
and unpads.
"""
import sys

sys.path.insert(0, "/opt/trn_rl_repo")

import numpy as np
import concourse.bacc as bacc
import concourse.mybir as mybir
import concourse.tile as tile
from concourse import bass_utils

P = 128
R = 782            # ceil(100000/128) -> node t = r*128 + q
RP = 896           # R padded to 7*128 for transposed blocks
NB_R = 7
N_CORES = 8
NUM_EDGES = 6_400_000
NUM_NODES = 100_000
EC = NUM_EDGES // N_CORES          # 800_000 edges per core
FC = EC // P                        # 6250 columns
FCP = 6256                          # padded columns: divisible by 8 and 4
B_G = 4                             # chunks per J-block, phase G (N=512)
NJB = FCP // B_G                    # 1564 J-blocks
TSLOT = (NJB + P - 1) // P          # 13 row-slots in the transposed layout
SUP = 16                            # J-blocks per select supertile (64 cols)

f16, f32 = mybir.dt.float16, mybir.dt.float32
_cache = {}


def _build_phase_s():
    nc = bacc.Bacc("TRN2", target_bir_lowering=False, debug=False,
                   enable_asserts=False)
    d_e = nc.dram_tensor("e", [P, FCP], f32, kind="ExternalInput")
    d_q = nc.dram_tensor("q", [P, FCP], f32, kind="ExternalInput")
    d_r = nc.dram_tensor("r", [P, FCP], f32, kind="ExternalInput")
    d_iota = nc.dram_tensor("iota", [P, RP], f16, kind="ExternalInput")
    d_T = nc.dram_tensor("T", [P, R], f32, kind="ExternalOutput")
    d_expe = nc.dram_tensor("expe", [P, FCP], f16, kind="ExternalOutput")
    OP = mybir.AluOpType

    with tile.TileContext(nc) as tc:
        with (
            tc.tile_pool(name="const", bufs=1) as cpool,
            tc.tile_pool(name="stage", bufs=1) as spool,
            tc.tile_pool(name="work", bufs=4) as wpool,
            tc.tile_pool(name="psum", bufs=1, space="PSUM") as ppool,
        ):
            iq = cpool.tile([P, P], f16)
            ir = cpool.tile([P, R], f16)
            nc.sync.dma_start(out=iq[:], in_=d_iota[:, 0:P])
            nc.sync.dma_start(out=ir[:], in_=d_iota[:, 0:R])

            e_sb = spool.tile([P, FCP], f32)
            q_sb = spool.tile([P, FCP], f32)
            r_sb = spool.tile([P, FCP], f32)
            expe = spool.tile([P, FCP], f32)
            expe16 = spool.tile([P, FCP], f16)
            nc.sync.dma_start(out=e_sb[:], in_=d_e[:])
            nc.sync.dma_start(out=q_sb[:], in_=d_q[:])
            nc.sync.dma_start(out=r_sb[:], in_=d_r[:])
            for c0 in range(0, FCP, 1564):
                c1 = min(c0 + 1564, FCP)
                nc.scalar.activation(expe[:, c0:c1], e_sb[:, c0:c1],
                                     mybir.ActivationFunctionType.Exp)
                nc.vector.tensor_copy(out=expe16[:, c0:c1], in_=expe[:, c0:c1])

            psumT = ppool.tile([P, R], f32, space="PSUM")
            for j in range(FCP):
                first = (j == 0)
                last = (j == FCP - 1)
                qexp_eq = wpool.tile([P, P], f16, tag="qexp_eq")
                qexp = wpool.tile([P, P], f16, tag="qexp")
                eqR = wpool.tile([P, R], f16, tag="eqR")
                nc.vector.tensor_scalar(
                    out=qexp_eq[:], in0=iq[:],
                    scalar1=q_sb[:, j:j + 1], scalar2=None,
                    op0=OP.is_equal)
                nc.scalar.activation(qexp[:], qexp_eq[:],
                                     mybir.ActivationFunctionType.Copy,
                                     scale=expe[:, j:j + 1])
                nc.vector.tensor_scalar(
                    out=eqR[:], in0=ir[:],
                    scalar1=r_sb[:, j:j + 1], scalar2=None,
                    op0=OP.is_equal)
                nc.tensor.matmul(out=psumT[:, 0:512],
                                 lhsT=qexp[:], rhs=eqR[:, 0:512],
                                 start=first, stop=last)
                nc.tensor.matmul(out=psumT[:, 512:R],
                                 lhsT=qexp[:], rhs=eqR[:, 512:R],
                                 start=first, stop=last)
            outT = spool.tile([P, R], f32)
            nc.vector.tensor_copy(out=outT[:], in_=psumT[:])
            nc.sync.dma_start(out=d_T[:], in_=outT[:])
            nc.sync.dma_start(out=d_expe[:], in_=expe16[:])
    nc.compile()
    return nc


def _build_phase_g():
    nc = bacc.Bacc("TRN2", target_bir_lowering=False, debug=False,
                   enable_asserts=False)
    d_Tall = nc.dram_tensor("Tall", [P, N_CORES, R], f32, kind="ExternalInput")
    d_qT = nc.dram_tensor("qT", [1, NJB * 512], f16, kind="ExternalInput")
    d_rT = nc.dram_tensor("rT", [1, NJB * 512], f16, kind="ExternalInput")
    d_expe = nc.dram_tensor("expe", [P, FCP], f16, kind="ExternalInput")
    d_id = nc.dram_tensor("ident", [P, P], f16, kind="ExternalInput")
    d_ones = nc.dram_tensor("ones", [P, P], f16, kind="ExternalInput")
    d_iotaPB = nc.dram_tensor("iotaPB", [P, NB_R], f32, kind="ExternalInput")
    d_alpha = nc.dram_tensor("alpha", [P, FCP], f32, kind="ExternalOutput")
    OP = mybir.AluOpType

    with tile.TileContext(nc) as tc:
        with (
            tc.tile_pool(name="const", bufs=1) as cpool,
            tc.tile_pool(name="stage", bufs=1) as spool,
            tc.tile_pool(name="work", bufs=4) as wpool,
            tc.tile_pool(name="strips", bufs=2) as stpool,
            tc.tile_pool(name="psum", bufs=2, space="PSUM") as ppool,
            tc.tile_pool(name="psumw", bufs=1, space="PSUM") as ppoolw,
        ):
            ident = cpool.tile([P, P], f16)
            ones = cpool.tile([P, P], f16)
            iPB = cpool.tile([P, NB_R], f32)
            nc.sync.dma_start(out=ident[:], in_=d_id[:])
            nc.sync.dma_start(out=ones[:], in_=d_ones[:])
            nc.sync.dma_start(out=iPB[:], in_=d_iotaPB[:])

            expe = spool.tile([P, FCP], f16)
            nc.sync.dma_start(out=expe[:], in_=d_expe[:])

            # On-device 8-way all-reduce of the local tables
            Tparts = spool.tile([P, N_CORES, R], f32)
            nc.sync.dma_start(out=Tparts[:], in_=d_Tall[:])
            Tsum = spool.tile([P, R], f32)
            nc.vector.tensor_tensor(out=Tsum[:], in0=Tparts[:, 0, :],
                                    in1=Tparts[:, 1, :], op=OP.add)
            for c in range(2, N_CORES):
                nc.vector.tensor_tensor(out=Tsum[:], in0=Tsum[:],
                                        in1=Tparts[:, c, :], op=OP.add)

            # W = min(1/(T + 1e-16), 6e4) in f16, zero-padded to RP
            W16 = spool.tile([P, RP], f16)
            Wf = spool.tile([P, R], f32)
            nc.vector.tensor_scalar_add(out=Wf[:], in0=Tsum[:], scalar1=1e-16)
            nc.vector.reciprocal(out=Wf[:], in_=Wf[:])
            nc.gpsimd.memset(W16[:], 0.0)
            nc.vector.tensor_scalar_min(out=W16[:, 0:R], in0=Wf[:], scalar1=60000.0)

            # WT_b [r', q] = W16[:, 128b:128(b+1)]^T
            WT = spool.tile([P, NB_R, P], f16)
            for b in range(NB_R):
                pt = ppoolw.tile([P, P], f16, space="PSUM", tag="psw")
                nc.tensor.transpose(out=pt[:], in_=W16[:, P * b:P * (b + 1)],
                                    identity=ident[:])
                nc.scalar.copy(out=WT[:, b, :], in_=pt[:])

            alpha_sb = spool.tile([P, FCP], f32)
            NW = B_G * P  # 512
            n_sup = (NJB + SUP - 1) // SUP
            for sup in range(n_sup):
                jb_lo = sup * SUP
                jb_hi = min(jb_lo + SUP, NJB)
                nsel = (jb_hi - jb_lo) * B_G
                psumW = ppoolw.tile([P, P], f32, space="PSUM", tag="psw")
                qstrip = stpool.tile([1, SUP * 512], f16, tag="qstrip")
                rstrip = stpool.tile([1, SUP * 512], f16, tag="rstrip")
                nc.sync.dma_start(out=qstrip[0:1, 0:nsel * P],
                                  in_=d_qT[0:1, jb_lo * 512:jb_lo * 512 + nsel * P])
                nc.sync.dma_start(out=rstrip[0:1, 0:nsel * P],
                                  in_=d_rT[0:1, jb_lo * 512:jb_lo * 512 + nsel * P])
                for jb in range(jb_lo, jb_hi):
                    j0 = jb * B_G
                    coff = (jb - jb_lo) * 512
                    # row-broadcasts via K=1 outer-product matmuls
                    p_rbc = ppool.tile([P, NW], f32, space="PSUM", tag="prbc")
                    p_qbc = ppool.tile([P, NW], f32, space="PSUM", tag="pqbc")
                    nc.tensor.matmul(
                        out=p_rbc[:], lhsT=ones[0:1, :],
                        rhs=rstrip[0:1, coff:coff + NW],
                        start=True, stop=True)
                    nc.tensor.matmul(
                        out=p_qbc[:], lhsT=ones[0:1, :],
                        rhs=qstrip[0:1, coff:coff + NW],
                        start=True, stop=True)
                    rbc = wpool.tile([P, NW], f16, tag="rbc")
                    qbc = wpool.tile([P, NW], f16, tag="qbc")
                    nc.scalar.copy(out=rbc[:], in_=p_rbc[:])
                    nc.scalar.copy(out=qbc[:], in_=p_qbc[:])
                    psumM = ppool.tile([P, NW], f32, space="PSUM", tag="psM")
                    eqRT = wpool.tile([P, NB_R, NW], f16, tag="eqRT")
                    for b in range(NB_R):
                        nc.vector.tensor_scalar(
                            out=eqRT[:, b, :], in0=rbc[:],
                            scalar1=iPB[:, b:b + 1], scalar2=None,
                            op0=OP.is_equal)
                        nc.tensor.matmul(out=psumM[:], lhsT=WT[:, b, :],
                                         rhs=eqRT[:, b, :],
                                         start=(b == 0), stop=(b == NB_R - 1))
                    Mcp = wpool.tile([P, NW], f16, tag="Mcp")
                    nc.scalar.copy(out=Mcp[:], in_=psumM[:])
                    MQ = wpool.tile([P, NW], f16, tag="MQ")
                    nc.vector.scalar_tensor_tensor(
                        out=MQ[:], in0=qbc[:], scalar=iPB[:, 0:1], in1=Mcp[:],
                        op0=OP.is_equal, op1=OP.mult)
                    for j in range(B_G):
                        col = (jb - jb_lo) * B_G + j
                        nc.tensor.matmul(out=psumW[:, col:col + 1],
                                         lhsT=MQ[:, j * P:(j + 1) * P],
                                         rhs=ones[:, 0:1], start=True, stop=True)
                c0 = jb_lo * B_G
                nc.vector.tensor_tensor(out=alpha_sb[:, c0:c0 + nsel],
                                        in0=expe[:, c0:c0 + nsel],
                                        in1=psumW[:, 0:nsel], op=OP.mult)
            nc.sync.dma_start(out=d_alpha[:], in_=alpha_sb[:])
    nc.compile()
    return nc


def _get_neffs():
    if "s" not in _cache:
        _cache["s"] = _build_phase_s()
    if "g" not in _cache:
        _cache["g"] = _build_phase_g()
    return _cache["s"], _cache["g"]


def _transposed_layout(mat16):
    """[128, FCP] f16 -> [1, NJB*512] f16 edge-major rows per J-block."""
    arr = mat16.reshape(P, NJB, B_G).transpose(1, 2, 0).reshape(1, NJB * B_G * P)
    return np.ascontiguousarray(arr)


def prep_inputs(e, edge_index):
    e = np.asarray(e, dtype=np.float32).reshape(-1)
    t = np.asarray(edge_index)[1].astype(np.int64)
    q = (t % P).astype(np.float32)
    r = (t // P).astype(np.float32)
    iota = np.arange(RP, dtype=np.float16)[None, :].repeat(P, axis=0)
    identity = np.eye(P, dtype=np.float16)
    ones_m = np.ones((P, P), dtype=np.float16)
    iotaPB = (np.arange(P, dtype=np.float32)[:, None]
              + 128.0 * np.arange(NB_R, dtype=np.float32)[None, :])
    pad = FCP - FC
    in_maps_s, qT_mats, rT_mats = [], [], []
    for c in range(N_CORES):
        sl = slice(c * EC, (c + 1) * EC)
        e_m = np.pad(e[sl].reshape(P, FC), ((0, 0), (0, pad)),
                     constant_values=-100.0)
        q_m = np.pad(q[sl].reshape(P, FC), ((0, 0), (0, pad)))
        r_m = np.pad(r[sl].reshape(P, FC), ((0, 0), (0, pad)))
        qT_mats.append(_transposed_layout(q_m.astype(np.float16)))
        rT_mats.append(_transposed_layout(r_m.astype(np.float16)))
        in_maps_s.append({"e": e_m, "q": q_m, "r": r_m, "iota": iota})
    consts = {"ident": identity, "ones": ones_m, "iotaPB": iotaPB}
    return in_maps_s, qT_mats, rT_mats, consts


def make_g_maps(res_s, qT_mats, rT_mats, consts):
    Tall = np.stack([res_s.results[c]["T"] for c in range(N_CORES)], axis=1)
    return [{"Tall": Tall, "qT": qT_mats[c], "rT": rT_mats[c],
             "expe": res_s.results[c]["expe"], **consts}
            for c in range(N_CORES)]


def kernel(e, edge_index, num_nodes):
    assert int(num_nodes) == NUM_NODES
    nc_s, nc_g = _get_neffs()
    in_maps_s, qT_mats, rT_mats, consts = prep_inputs(e, edge_index)
    res_s = bass_utils.run_bass_kernel_spmd(nc_s, in_maps_s,
                                            core_ids=list(range(N_CORES)))
    in_maps_g = make_g_maps(res_s, qT_mats, rT_mats, consts)
    res_g = bass_utils.run_bass_kernel_spmd(nc_g, in_maps_g,
                                            core_ids=list(range(N_CORES)))
    alpha = np.empty(NUM_EDGES, dtype=np.float32)
    for c in range(N_CORES):
        alpha[c * EC:(c + 1) * EC] = \
            res_g.results[c]["alpha"][:, :FC].reshape(-1)
    return alpha



# revision 2
# speedup vs baseline: 5.5202x; 5.5202x over previous
"""Segment-softmax (GAT stage 4) Trainium2 kernel, 8 NeuronCores.

alpha_i = exp(e_i) / (sum_{j: tgt_j = tgt_i} exp(e_j) + 1e-16)

Design (edge-parallel, bucket-sorted):
  - Edges sharded across 8 cores (800k each). Node t factored t = r*32 + q
    with q in [0,32), r in [0,3200) buckets (25 blocks of 128).
  - Host counting-sorts each core's edges by bucket r (pure layout permute)
    into a COMMON column layout: buckets grouped 32-per-group, each group's
    buckets padded to the same chunk count (max over cores/buckets), so one
    SPMD NEFF serves all cores. Each 128-edge chunk is r-pure.
  - Phase S (histogram): DVE builds one-hot Q[e,q] = (q_e == q) batched per
    group; PE accumulates T[q, r] += Q_chunk^T @ expe_col per chunk into a
    PSUM-resident [32, 3200] table (start/stop per bucket). Matmul rhs is
    the exp(e) column, so the one-hot stays exact in f16 and each chunk
    costs only a 32-column LDWEIGHTS + 1-cycle matmul.
  - Host relays the 8 local tables; Phase G sums them on-device, computes
    W = min(1/(T+1e-16), 6e4), transposes W via PE into a t-ordered flat
    DRAM scratch, then per 32-bucket group broadcast-reads the [1,1024]
    W slice to all 128 partitions via DMA. DVE rebuilds the one-hot,
    multiplies by the broadcast W and max-reduces over q (exact: one-hot
    has a single nonzero) to produce per-edge w = W[t_e]; alpha = expe * w.
"""
import sys

sys.path.insert(0, "/opt/trn_rl_repo")

import numpy as np
import concourse.bacc as bacc
import concourse.mybir as mybir
import concourse.tile as tile
from concourse import bass_utils
from concourse.ap import AP as APC

P = 128
MQ = 32                  # t = r*MQ + q
NB = 3200                # bucket capacity (>= ceil(100000/32)=3125), 25*128
NBLK = NB // P           # 25 transpose blocks
GB = 32                  # buckets per group
NGRP = NB // GB          # 100
N_CORES = 8
NUM_EDGES = 6_400_000
NUM_NODES = 100_000
EC = NUM_EDGES // N_CORES

f16, f32 = mybir.dt.float16, mybir.dt.float32
_cache = {}


def _build_s(cnt, C):
    nc = bacc.Bacc("TRN2", target_bir_lowering=False, debug=False,
                   enable_asserts=False)
    d_e = nc.dram_tensor("e", [P, C], f32, kind="ExternalInput")
    d_q = nc.dram_tensor("q", [P, C], f16, kind="ExternalInput")
    d_iota = nc.dram_tensor("iota", [P, MQ], f16, kind="ExternalInput")
    d_T = nc.dram_tensor("T", [MQ, NB], f32, kind="ExternalOutput")
    d_expe = nc.dram_tensor("expe", [P, C], f16, kind="ExternalOutput")
    OP = mybir.AluOpType

    with tile.TileContext(nc) as tc:
        with (
            tc.tile_pool(name="const", bufs=1) as cpool,
            tc.tile_pool(name="stage", bufs=1) as spool,
            tc.tile_pool(name="etmp", bufs=2) as epool,
            tc.tile_pool(name="work", bufs=3) as wpool,
            tc.tile_pool(name="psum", bufs=1, space="PSUM") as ppool,
        ):
            iq = cpool.tile([P, MQ], f16)
            nc.sync.dma_start(out=iq[:], in_=d_iota[:])
            q_sb = spool.tile([P, C], f16)
            e_sb = spool.tile([P, C], f32)
            expe16 = spool.tile([P, C], f16)
            nc.sync.dma_start(out=q_sb[:], in_=d_q[:])
            nc.sync.dma_start(out=e_sb[:], in_=d_e[:])
            STRIP = 2048
            for s0 in range(0, C, STRIP):
                s1 = min(s0 + STRIP, C)
                etmp = epool.tile([P, STRIP], f32, tag="etmp")
                nc.scalar.activation(etmp[:, 0:s1 - s0], e_sb[:, s0:s1],
                                     mybir.ActivationFunctionType.Exp)
                nc.vector.tensor_copy(out=expe16[:, s0:s1],
                                      in_=etmp[:, 0:s1 - s0])

            psumT = ppool.tile([MQ, NB], f32, space="PSUM")
            col = 0
            for g in range(NGRP):
                k = int(cnt[g])
                ncols = GB * k
                Qg = wpool.tile([P, GB, k, MQ], f16, tag="Qg")
                in0 = (q_sb[:, col:col + ncols]
                       .rearrange("p (b k) -> p b k", b=GB)[:, :, :, None]
                       .broadcast_to([P, GB, k, MQ]))
                in1 = iq[:][:, None, None, :].broadcast_to([P, GB, k, MQ])
                nc.vector.tensor_tensor(out=Qg[:], in0=in0, in1=in1,
                                        op=OP.is_equal)
                for b in range(GB):
                    rr = g * GB + b
                    for kk in range(k):
                        c = col + b * k + kk
                        nc.tensor.matmul(out=psumT[:, rr:rr + 1],
                                         lhsT=Qg[:, b, kk, :],
                                         rhs=expe16[:, c:c + 1],
                                         start=(kk == 0), stop=(kk == k - 1))
                col += ncols
            outT = spool.tile([MQ, NB], f32)
            nc.vector.tensor_copy(out=outT[:], in_=psumT[:])
            nc.sync.dma_start(out=d_T[:], in_=outT[:])
            nc.sync.dma_start(out=d_expe[:], in_=expe16[:])
    nc.compile()
    return nc


def _build_g(cnt, C):
    nc = bacc.Bacc("TRN2", target_bir_lowering=False, debug=False,
                   enable_asserts=False)
    d_Tall = nc.dram_tensor("Tall", [MQ, N_CORES, NB], f32,
                            kind="ExternalInput")
    d_q = nc.dram_tensor("q", [P, C], f16, kind="ExternalInput")
    d_expe = nc.dram_tensor("expe", [P, C], f16, kind="ExternalInput")
    d_iota = nc.dram_tensor("iota", [P, MQ], f16, kind="ExternalInput")
    d_ident = nc.dram_tensor("ident", [MQ, MQ], f16, kind="ExternalInput")
    d_WT = nc.dram_tensor("WT", [P, NBLK * MQ], f16, kind="ExternalOutput")
    d_alpha = nc.dram_tensor("alpha", [P, C], f32, kind="ExternalOutput")
    OP = mybir.AluOpType

    with tile.TileContext(nc) as tc:
        with (
            tc.tile_pool(name="const", bufs=1) as cpool,
            tc.tile_pool(name="stage", bufs=1) as spool,
            tc.tile_pool(name="ttmp", bufs=2) as tpool,
            tc.tile_pool(name="wbc", bufs=3) as wbpool,
            tc.tile_pool(name="work", bufs=2) as wpool,
            tc.tile_pool(name="alph", bufs=2) as apool,
            tc.tile_pool(name="pt", bufs=2, space="PSUM") as ptpool,
        ):
            iq = cpool.tile([P, MQ], f16)
            ident = cpool.tile([MQ, MQ], f16)
            nc.sync.dma_start(out=iq[:], in_=d_iota[:])
            nc.sync.dma_start(out=ident[:], in_=d_ident[:])
            q_sb = spool.tile([P, C], f16)
            expe16 = spool.tile([P, C], f16)
            nc.sync.dma_start(out=q_sb[:], in_=d_q[:])
            nc.sync.dma_start(out=expe16[:], in_=d_expe[:])

            # on-device 8-way table all-reduce
            Tacc = spool.tile([MQ, NB], f32)
            nc.sync.dma_start(out=Tacc[:], in_=d_Tall[:, 0, :])
            for c in range(1, N_CORES):
                tmp = tpool.tile([MQ, NB], f32, tag="tt")
                nc.sync.dma_start(out=tmp[:], in_=d_Tall[:, c, :])
                nc.vector.tensor_tensor(out=Tacc[:], in0=Tacc[:],
                                        in1=tmp[:], op=OP.add)
            nc.vector.tensor_scalar_add(out=Tacc[:], in0=Tacc[:],
                                        scalar1=1e-16)
            nc.vector.reciprocal(out=Tacc[:], in_=Tacc[:])
            W16 = spool.tile([MQ, NB], f16)
            nc.vector.tensor_scalar_min(out=W16[:], in0=Tacc[:],
                                        scalar1=60000.0)

            # WT[r_lo, blk, q] = W16[q, blk*128 + r_lo], flat t-order per blk
            WT_sb = spool.tile([P, NBLK, MQ], f16)
            for bk in range(NBLK):
                pt = ptpool.tile([P, MQ], f16, space="PSUM", tag="pt")
                nc.tensor.transpose(out=pt[:], in_=W16[:, bk * P:(bk + 1) * P],
                                    identity=ident[:])
                nc.scalar.copy(out=WT_sb[:, bk, :], in_=pt[:])
            nc.sync.dma_start(out=d_WT[:], in_=WT_sb[:])

            col = 0
            for g in range(NGRP):
                k = int(cnt[g])
                ncols = GB * k
                # broadcast-read this group's 1024 W values to all partitions
                Wbc = wbpool.tile([P, GB, MQ], f16, tag="wbc")
                off0 = (GB * (g % 4)) * (NBLK * MQ) + (g // 4) * MQ
                src = APC(d_WT[:].tensor, off0,
                          [[0, P], [NBLK * MQ, GB], [1, MQ]])
                nc.sync.dma_start(out=Wbc[:], in_=src)

                Qg = wpool.tile([P, GB, k, MQ], f16, tag="Qg")
                in0 = (q_sb[:, col:col + ncols]
                       .rearrange("p (b k) -> p b k", b=GB)[:, :, :, None]
                       .broadcast_to([P, GB, k, MQ]))
                in1 = iq[:][:, None, None, :].broadcast_to([P, GB, k, MQ])
                nc.vector.tensor_tensor(out=Qg[:], in0=in0, in1=in1,
                                        op=OP.is_equal)
                Pg = wpool.tile([P, GB, k, MQ], f16, tag="Pg")
                nc.vector.tensor_tensor(
                    out=Pg[:], in0=Qg[:],
                    in1=Wbc[:][:, :, None, :].broadcast_to([P, GB, k, MQ]),
                    op=OP.mult)
                wg = wpool.tile([P, GB * k], f16, tag="wg")
                nc.vector.tensor_reduce(
                    out=wg[:], in_=Pg[:], axis=mybir.AxisListType.X,
                    op=OP.max)
                alph = apool.tile([P, GB * k], f32, tag="al")
                nc.vector.tensor_tensor(out=alph[:, 0:ncols],
                                        in0=expe16[:, col:col + ncols],
                                        in1=wg[:, 0:ncols], op=OP.mult)
                nc.sync.dma_start(out=d_alpha[:, col:col + ncols],
                                  in_=alph[:, 0:ncols])
                col += ncols
    nc.compile()
    return nc


def _get_neffs(meta):
    key = (meta["C"], meta["cnt"].tobytes())
    if key not in _cache:
        _cache[key] = (_build_s(meta["cnt"], meta["C"]),
                       _build_g(meta["cnt"], meta["C"]))
    return _cache[key]


def prep_inputs(e, edge_index):
    e = np.asarray(e, dtype=np.float32).reshape(-1)
    t = np.asarray(edge_index)[1].astype(np.int64)
    r_all = (t // MQ).astype(np.int32).reshape(N_CORES, EC)
    q_all = (t % MQ).astype(np.int32).reshape(N_CORES, EC)
    e_all = e.reshape(N_CORES, EC)

    # common layout: per-bucket chunk need = ceil(max-core-count/128),
    # padded to the max within each 32-bucket group
    need = np.zeros(NB, np.int64)
    counts = np.empty((N_CORES, NB), np.int64)
    for c in range(N_CORES):
        counts[c] = np.bincount(r_all[c], minlength=NB)
    need = -(-counts.max(axis=0) // P)
    cnt = np.maximum(need.reshape(NGRP, GB).max(axis=1), 1).astype(np.int32)
    C = int((GB * cnt).sum())
    gbase = np.concatenate([[0], np.cumsum(GB * cnt)])[:-1]
    # base column of each bucket
    base = np.repeat(gbase, GB) + (np.arange(NB) % GB) * np.repeat(cnt, GB)

    iota = np.arange(MQ, dtype=np.float16)[None, :].repeat(P, axis=0)
    ident = np.eye(MQ, dtype=np.float16)

    in_maps_s, placements = [], []
    for c in range(N_CORES):
        r, q, ec = r_all[c], q_all[c], e_all[c]
        order = np.argsort(r, kind="stable")
        rs = r[order]
        starts = np.concatenate([[0], np.cumsum(counts[c])])
        rank = np.arange(EC, dtype=np.int64) - starts[rs]
        part = (rank % P).astype(np.int32)
        colp = (base[rs] + rank // P).astype(np.int32)
        e_pad = np.full((P, C), -100.0, np.float32)
        q16 = np.zeros((P, C), np.float16)
        e_pad[part, colp] = ec[order]
        q16[part, colp] = q[order].astype(np.float16)
        in_maps_s.append({"e": e_pad, "q": q16, "iota": iota})
        placements.append((order, part, colp))

    meta = {"C": C, "cnt": cnt, "placements": placements,
            "iota": iota, "ident": ident}
    return in_maps_s, meta


def make_g_maps(res_s, meta):
    Tall = np.stack([res_s.results[c]["T"] for c in range(N_CORES)], axis=1)
    maps = []
    for c in range(N_CORES):
        maps.append({"Tall": Tall, "q": None, "expe": res_s.results[c]["expe"],
                     "iota": meta["iota"], "ident": meta["ident"]})
    return maps


def unpack_alpha(res_g, meta):
    alpha = np.empty(NUM_EDGES, dtype=np.float32)
    for c in range(N_CORES):
        order, part, colp = meta["placements"][c]
        a = res_g.results[c]["alpha"]
        shard = np.empty(EC, dtype=np.float32)
        shard[order] = a[part, colp]
        alpha[c * EC:(c + 1) * EC] = shard
    return alpha


def kernel(e, edge_index, num_nodes):
    assert int(num_nodes) == NUM_NODES
    in_maps_s, meta = prep_inputs(e, edge_index)
    nc_s, nc_g = _get_neffs(meta)
    res_s = bass_utils.run_bass_kernel_spmd(nc_s, in_maps_s,
                                            core_ids=list(range(N_CORES)))
    in_maps_g = make_g_maps(res_s, meta)
    for c in range(N_CORES):
        in_maps_g[c]["q"] = in_maps_s[c]["q"]
    res_g = bass_utils.run_bass_kernel_spmd(nc_g, in_maps_g,
                                            core_ids=list(range(N_CORES)))
    return unpack_alpha(res_g, meta)
